# revision 76
# baseline (speedup 1.0000x reference)
"""Fused attention-encoding kernel for Trainium2, 8-core batch-parallel SPMD.

Problem (per batch b of 16, p=1024 tokens, d=512 features):
    A[i,j] = wa.P_i + wb.P_j + (wc*P_i).P_j        (si = wa.P_i cancels in softmax)
    SA     = softmax_j(A)
    attn   = SA @ P
    Pc     = [P, attn]
    out    = sigmoid(Pc@w2) * P + sigmoid(Pc@w3) * tanh(Pc@w1)

Strategy: batch-parallel over 8 cores (2 batches/core). Scores are computed
transposed (S^T[j,i], j on partitions) so sj folds into the exp as a
per-partition activation bias and the attention matmul consumes E=exp(S^T)
directly. The score/attention/rowsum matmuls run in fp8-e4m3 DoubleRow mode
(2 k-tiles per instruction); softmax protects them from quantization noise.
The gate matmuls are precision-graduated per gate (see GATE_MODE): the tanh
z-gate keeps half its P-contraction in bf16, the sigmoid gates run fully in
fp8 DoubleRow. All transposes, scale folds, and sj = P@wb are precomputed
host-side; inputs are shipped in SBUF-layout
([128 partitions, free]) so each tensor is one or two straight DMAs (DMA
triggers cost ~0.6us each on an engine ring, so fewer is faster). The softmax
reciprocal chain avoids the slow gpsimd partition-broadcast by broadcasting
the rowsum via a K=1 float32r matmul (full fp32 precision at bf16 speed).

Scale scheme (fp8-e4m3 wants operands ~O(1)):
    pwt8 = (P * wc * 32)^T   -> score PSUM is 32x, exp uses scale=1/32, bias=sjT
    ones = 1/8               -> rowsum PSUM = rs/8, so rb32 = 8/rs
    at8  = attn_unnorm * rb32 = 8*attn (fp8)
    pt16 = (P*32)^T bf16, w16 plain; w8 = w[512:]*4  -> gate PSUM is 32x logits,
    activations use scale=1/32 (bias b*32 added to PSUM before the rescale).
"""

import sys

if "/opt/trn_rl_repo" not in sys.path:
    sys.path.insert(0, "/opt/trn_rl_repo")

from contextlib import ExitStack

import ml_dtypes
import numpy as np

import concourse.bass as bass
import concourse.mybir as mybir
import concourse.tile as tile
from concourse import bacc
from concourse.bass_utils import run_bass_kernel_spmd

B, PL, D = 16, 1024, 512
NCORES = 8
BPC = B // NCORES          # batches per core
NI = PL // 128             # token blocks (i or j): 8
ND = D // 128              # feature chunks: 4
FP32 = mybir.dt.float32
FP32R = mybir.dt.float32r
BF16 = mybir.dt.bfloat16
FP8 = mybir.dt.float8e4
AF = mybir.ActivationFunctionType
DR = mybir.MatmulPerfMode.DoubleRow

NPF8 = ml_dtypes.float8_e4m3
NPBF = ml_dtypes.bfloat16

import os

# Per-gate P-half precision: how many of the 4 contraction chunks run in bf16
# (the rest run as fp8 DoubleRow pairs). The tanh z-gate amplifies logit error
# ~4x more than the sigmoids, so it keeps the bf16 chunks:
#   safe  (2,2,2): rel_err 1.12e-2   zsafe (2,0,0): 1.25e-2   full (0,0,0): 1.58e-2
GATE_MODE = os.environ.get("K_GATE_MODE", "zsafe")
GATE_BF16 = {"safe": (2, 2, 2), "zsafe": (2, 0, 0), "full": (0, 0, 0)}[GATE_MODE]
# per-gate chunk counts in w8 ([4-nbf P-chunks] + [4 attn chunks]) and offsets
W8_NCH = [8 - nbf for nbf in GATE_BF16]
W8_OFF = [sum(W8_NCH[:g]) for g in range(3)]
W16_OFF = [sum(GATE_BF16[:g]) for g in range(3)]
NW16 = sum(GATE_BF16)
NW8 = sum(W8_NCH)

_cache = {}


def _build(with_bias: bool, taps: tuple = ()):
    nc = bacc.Bacc(
        "TRN2", target_bir_lowering=False, debug=False, num_devices=1
    )
    # pt8/pwt8 packed as one tensor in chunk order [pt01, pwt01, pt23, pwt23]
    # so scores jb0's full operand set is one leading DMA (triggers ~0.6us each)
    pq8_d = nc.dram_tensor("pq8", [BPC, 128, 2 * ND * PL], FP8, kind="ExternalInput").ap()
    if NW16:
        pt16_d = nc.dram_tensor("pt16", [BPC, 128, 2 * PL], BF16, kind="ExternalInput").ap()
    pn8_d = nc.dram_tensor("pn8", [BPC, 128, NI * D], FP8, kind="ExternalInput").ap()
    pn32_d = nc.dram_tensor("pn32", [BPC, 128, NI * D], FP32, kind="ExternalInput").ap()
    sjt_d = nc.dram_tensor("sjt", [BPC, 128, NI], FP32, kind="ExternalInput").ap()
    if NW16:
        w16_d = nc.dram_tensor("w16", [128, NW16 * D], BF16, kind="ExternalInput").ap()
    w8_d = nc.dram_tensor("w8", [128, NW8 * D], FP8, kind="ExternalInput").ap()
    if with_bias:
        b_d = nc.dram_tensor("b32", [3, D], FP32, kind="ExternalInput").ap()
    out_d = nc.dram_tensor("out", [BPC, PL, D], FP32, kind="ExternalOutput").ap()
    tap_d = {}

    def tap(name, ap, lb=0):
        if lb != 0 or name not in taps:
            return
        t = nc.dram_tensor(
            f"tap_{name}", list(ap.shape), ap.dtype, kind="ExternalOutput"
        ).ap()
        tap_d[name] = t
        nc.sync.dma_start(t, ap)

    with tile.TileContext(nc) as tc, ExitStack() as ctx:
        pool = lambda name, bufs: ctx.enter_context(
            tc.tile_pool(name=name, bufs=bufs)
        )
        const = pool("const", 1)
        wpool = pool("wts", 1)
        pt8p = pool("pt8", 2)
        pt16p = pool("pt16", 2)
        pn8p = pool("pn8", 2)
        pn32p = pool("pn32", 2)
        e8p = pool("e8", 2)
        at8p = pool("at8", 2)
        rb32p = pool("rb32", 2)
        smallp = pool("small", 2)
        gp = pool("gates", 2)
        tmpp = pool("tmp", 2)
        op = pool("outs", 3)
        psmm = ctx.enter_context(tc.tile_pool(name="psmm", bufs=6, space="PSUM"))
        psvec = ctx.enter_context(tc.tile_pool(name="psvec", bufs=2, space="PSUM"))

        # --- constants / weights (loaded once, after batch-0 critical loads) ---
        if NW16:
            w16_sb = wpool.tile([128, NW16 * D], BF16, tag="w16")
        w8_sb = wpool.tile([128, NW8 * D], FP8, tag="w8")

        def load_weights():
            if NW16:
                nc.sync.dma_start(w16_sb[:], w16_d)
            nc.sync.dma_start(w8_sb[:], w8_d)

        # DoubleRow ldweights needs the k-tile pair step %16==0, so space the
        # two ones columns 16 elements apart.
        ones8 = const.tile([128, 32], FP8, tag="ones8")
        nc.vector.memset(ones8[:], 0.125)
        ones16 = const.tile([1, 128], BF16, tag="ones16")
        nc.vector.memset(ones16[:], 1.0)
        if with_bias:
            bb = [const.tile([128, D], FP32, tag=f"bias{g}", name=f"bias{g}") for g in range(3)]
            btmp = const.tile([1, 3 * D], FP32, tag="btmp")
            nc.sync.dma_start(btmp[:], b_d.rearrange("g e -> (g e)")[None, :])
            for g in range(3):
                nc.gpsimd.partition_broadcast(
                    bb[g][:], btmp[0:1, g * D : (g + 1) * D]
                )

        for lb in range(BPC):
            # ---------- phase A: loads (ring order = HBM priority) ----------
            pq8 = pt8p.tile([128, 2 * ND * PL], FP8, tag="pq8")
            for sl in (slice(0, 4 * PL), slice(4 * PL, 8 * PL)):
                nc.sync.dma_start(pq8[:, sl], pq8_d[lb][:, sl])
            sjt = smallp.tile([128, NI], FP32, tag="sjt")
            nc.scalar.dma_start(sjt[:], sjt_d[lb])
            pn8 = pn8p.tile([128, NI * D], FP8, tag="pn8")
            nc.sync.dma_start(pn8[:], pn8_d[lb])
            if NW16:
                pt16 = pt16p.tile([128, 2 * PL], BF16, tag="pt16")
                nc.sync.dma_start(pt16[:], pt16_d[lb])
            if lb == 0:
                load_weights()
            pn32 = pn32p.tile([128, NI * D], FP32, tag="pn32")
            nc.sync.dma_start(pn32[:], pn32_d[lb])

            pq8v = pq8.rearrange("p (c l) -> p c l", l=PL)
            PT_SLOT = (0, 4)  # packed slot of pt8 chunk-pair dp
            PW_SLOT = (2, 6)  # packed slot of pwt8 chunk-pair dp
            pn8v = pn8.rearrange("p (j d) -> p j d", d=D)

            # ---------- phase B: scores (fp8 DR) + exp + rowsum (fp8 DR) ----------
            e8 = e8p.tile([128, NI * PL], FP8, tag="e8")
            e8v = e8.rearrange("p (j l) -> p j l", l=PL)
            ps_rs = [
                psvec.tile([128, 512], FP32, tag="psvec", name=f"psrs{lb}_{_}")
                for _ in range(2)
            ]

            def rowsum(jb, start, stop):
                for ih in range(2):
                    nc.tensor.matmul(
                        ps_rs[ih][0:1, :],
                        ones8[:, 0:17:16][:, :, None],
                        e8v[:, jb - 1 : jb + 1, ih * 512 : (ih + 1) * 512],
                        start=start,
                        stop=stop,
                        perf_mode=DR,
                    )

            for jb in range(NI):
                ps_s = [
                    psmm.tile([128, 512], FP32, tag="psmm", name=f"pss{lb}_{jb}_{_}")
                    for _ in range(2)
                ]
                for ih in range(2):
                    for dp in range(2):
                        nc.tensor.matmul(
                            ps_s[ih],
                            pq8v[:, PT_SLOT[dp] : PT_SLOT[dp] + 2, jb * 128 : (jb + 1) * 128],
                            pq8v[:, PW_SLOT[dp] : PW_SLOT[dp] + 2, ih * 512 : (ih + 1) * 512],
                            start=(dp == 0),
                            stop=(dp == 1),
                            perf_mode=DR,
                        )
                for ih in range(2):
                    nc.scalar.activation(
                        e8v[:, jb, ih * 512 : (ih + 1) * 512],
                        ps_s[ih][:],
                        AF.Exp,
                        bias=sjt[:, jb : jb + 1],
                        scale=1.0 / 32.0,
                    )
                if jb % 2 == 1 and jb < NI - 1:
                    rowsum(jb, start=(jb == 1), stop=False)

            # ---------- phase C: attn^T (fp8 DR) + normalize (-> 8*attn fp8) ----------
            # dc0's first 3 jc-pairs only need exps jb0-5, so they run while
            # exp jb6/jb7 drain; the last rowsum pair and the final dc0 matmul
            # wait on exp jb7. The rowsum broadcast (K=1 fp32r matmul) slots in
            # right after so the reciprocal chain overlaps attn dc1-dc3.
            at8 = at8p.tile([128, ND * PL], FP8, tag="at8")
            at8v = at8.rearrange("p (c l) -> p c l", l=PL)
            rs16 = smallp.tile([1, PL], BF16, tag="rs16")
            rb32 = rb32p.tile([128, PL], FP32, tag="rb32")
            ps_bc = []

            def attn_mm(dc, ih, jp, ps_a):
                nc.tensor.matmul(
                    ps_a[ih],
                    pn8v[:, 2 * jp : 2 * jp + 2, dc * 128 : (dc + 1) * 128],
                    e8v[:, 2 * jp : 2 * jp + 2, ih * 512 : (ih + 1) * 512],
                    start=(jp == 0),
                    stop=(jp == 3),
                    perf_mode=DR,
                )

            # attn dc0/dc1 jc-pairs 0-2 need only exps jb0-5, so they run while
            # exp jb6/jb7 drain; the last rowsum pair and the final jc-pairs
            # wait on exp jb7. The rowsum broadcast (K=1 bf16 matmul) slots in
            # right after so the reciprocal chain overlaps attn dc2/dc3.
            ps_a = {}
            for dc in range(2):
                ps_a[dc] = [
                    psmm.tile([128, 512], FP32, tag="psmm", name=f"psa{lb}_{dc}_{_}")
                    for _ in range(2)
                ]
                for ih in range(2):
                    for jp in range(3):
                        attn_mm(dc, ih, jp, ps_a[dc])
            rowsum(NI - 1, start=False, stop=True)
            for ih in range(2):
                nc.scalar.copy(rs16[0:1, ih * 512 : (ih + 1) * 512], ps_rs[ih][0:1, :])
            for dc in range(2):
                for ih in range(2):
                    attn_mm(dc, ih, 3, ps_a[dc])
            for ih in range(2):
                bc = psvec.tile([128, 512], FP32, tag="psvec", name=f"psbc{lb}_{ih}")
                ps_bc.append(bc)
                nc.tensor.matmul(
                    bc[:],
                    ones16[:],
                    rs16[0:1, ih * 512 : (ih + 1) * 512],
                    start=True,
                    stop=True,
                )
            # interleave recip/normalize per half so the dc0 norm (which gates
            # PSUM reuse for attn dc3) lands one recip earlier
            for ih in range(2):
                nc.vector.reciprocal_approx_fast(
                    out=rb32[:, ih * 512 : (ih + 1) * 512], in_=ps_bc[ih][:]
                )
                nc.vector.tensor_mul(
                    at8v[:, 0, ih * 512 : (ih + 1) * 512],
                    ps_a[0][ih][:],
                    rb32[:, ih * 512 : (ih + 1) * 512],
                )
            for ih in range(2):
                nc.vector.tensor_mul(
                    at8v[:, 1, ih * 512 : (ih + 1) * 512],
                    ps_a[1][ih][:],
                    rb32[:, ih * 512 : (ih + 1) * 512],
                )
            for dc in range(2, ND):
                # dc3 draws its PSUM from the psvec ring (freed by the recip)
                # instead of recycling dc0's psmm banks, which would make it
                # wait on the at8-norm at the end of the reciprocal chain
                pl_, tag_ = (psmm, "psmm") if dc == 2 else (psvec, "psvec")
                ps_ad = [
                    pl_.tile([128, 512], FP32, tag=tag_, name=f"psa{lb}_{dc}_{_}")
                    for _ in range(2)
                ]
                for ih in range(2):
                    for jp in range(4):
                        attn_mm(dc, ih, jp, ps_ad)
                for ih in range(2):
                    nc.vector.tensor_mul(
                        at8v[:, dc, ih * 512 : (ih + 1) * 512],
                        ps_ad[ih][:],
                        rb32[:, ih * 512 : (ih + 1) * 512],
                    )

            tap("sjt", sjt[:], lb)
            tap("e8", e8[:], lb)
            tap("rs32", rs16[:], lb)
            tap("at8", at8[:], lb)
            tap("w8_0", w8_sb[:, 0:2048], lb)

            # ---------- phase D: gates ----------
            # contraction: P chunks 0-1 bf16 (x32 P vs plain w), P chunks 2-3
            # as one fp8 DR pair (P vs 32w), attn chunks as two fp8 DR pairs
            # (8*attn vs 4w) -- every path lands 32x logits in PSUM.
            if NW16:
                pt16v = pt16.rearrange("p (c l) -> p c l", l=PL)
                w16v = w16_sb.rearrange("p (c d) -> p c d", d=D)
            w8v = w8_sb.rearrange("p (c d) -> p c d", d=D)
            for ib in range(NI):
                last = lb == BPC - 1 and ib == NI - 1
                ps_g = [
                    psmm.tile([128, 512], FP32, tag="psmm", name=f"psg{lb}_{ib}_{_}")
                    for _ in range(3)
                ]
                # final tile: z-gate matmuls last, so its PSUM stops right at
                # the end while r/f activations already ran -> shortest tail
                for g in ((1, 2, 0) if last else (0, 1, 2)):
                    nbf = GATE_BF16[g]
                    for dc in range(nbf):
                        nc.tensor.matmul(
                            ps_g[g],
                            pt16v[:, dc, ib * 128 : (ib + 1) * 128],
                            w16v[:, W16_OFF[g] + dc, :],
                            start=(dc == 0),
                            stop=False,
                        )
                    for dp in range(nbf // 2, 2):
                        nc.tensor.matmul(
                            ps_g[g],
                            pq8v[:, PT_SLOT[dp] : PT_SLOT[dp] + 2, ib * 128 : (ib + 1) * 128],
                            w8v[:, W8_OFF[g] + 2 * dp - nbf : W8_OFF[g] + 2 * dp - nbf + 2, :],
                            start=(nbf == 0 and dp == 0),
                            stop=False,
                            perf_mode=DR,
                        )
                    a0 = W8_OFF[g] + 4 - nbf
                    for cp in range(2):
                        nc.tensor.matmul(
                            ps_g[g],
                            at8v[:, 2 * cp : 2 * cp + 2, ib * 128 : (ib + 1) * 128],
                            w8v[:, a0 + 2 * cp : a0 + 2 + 2 * cp, :],
                            start=False,
                            stop=(cp == 1),
                            perf_mode=DR,
                        )
                if with_bias:
                    # bb holds b*32 so one 1/32 activation rescale covers both
                    for g in range(3):
                        nc.vector.tensor_add(ps_g[g][:], ps_g[g][:], bb[g][:])
                z32 = gp.tile([128, D], FP32, tag="z32")
                r32 = gp.tile([128, D], FP32, tag="r32")
                f32 = gp.tile([128, D], FP32, tag="f32")
                o32 = op.tile([128, D], FP32, tag="o32")
                t32 = tmpp.tile([128, D], FP32, tag="t32")
                sc = 1.0 / 32.0
                if not last:
                    # r first: the output chain is r -> o32 -> add -> DMA
                    nc.scalar.activation(r32[:], ps_g[1][:], AF.Sigmoid, scale=sc)
                    nc.scalar.activation(z32[:], ps_g[0][:], AF.Tanh, scale=sc)
                    nc.scalar.activation(f32[:], ps_g[2][:], AF.Sigmoid, scale=sc)
                    nc.vector.tensor_mul(o32[:], r32[:], pn32[:, ib * D : (ib + 1) * D])
                    # gpsimd offloads the f*z product off the vector engine
                    nc.gpsimd.tensor_mul(t32[:], f32[:], z32[:])
                    nc.vector.tensor_add(o32[:], o32[:], t32[:])
                    nc.gpsimd.dma_start(out_d[lb, ib * 128 : (ib + 1) * 128, :], o32[:])
                else:
                    # final tile: r/f acts + o32 overlap the z-gate matmuls;
                    # only z -> t -> add trails the last matmul. DMA triggers
                    # from the scalar ring so the slow gpsimd drain starts early
                    nc.scalar.activation(r32[:], ps_g[1][:], AF.Sigmoid, scale=sc)
                    nc.scalar.activation(f32[:], ps_g[2][:], AF.Sigmoid, scale=sc)
                    nc.scalar.activation(z32[:], ps_g[0][:], AF.Tanh, scale=sc)
                    nc.vector.tensor_mul(o32[:], r32[:], pn32[:, ib * D : (ib + 1) * D])
                    nc.vector.tensor_mul(t32[:], f32[:], z32[:])
                    nc.vector.tensor_add(o32[:], o32[:], t32[:])
                    nc.scalar.dma_start(out_d[lb, ib * 128 : (ib + 1) * 128, :], o32[:])

    nc.compile()
    return nc


def _get_nc(with_bias: bool, taps: tuple = ()):
    key = (with_bias, taps)
    if key not in _cache:
        _cache[key] = _build(with_bias, taps)
    return _cache[key]


def _prep_in_maps(P, w_atten, w1, w2, w3, b1, b2, b3):
    P = np.ascontiguousarray(np.asarray(P, dtype=np.float32))
    w_atten = np.asarray(w_atten, dtype=np.float32)
    wb = w_atten[D : 2 * D]
    wc = w_atten[2 * D :]

    # transposed layouts [B, 128, ND*PL]: arr[b, p, c*PL+l] = P[b, l, c*128+p]
    PT = np.ascontiguousarray(
        P.reshape(B, PL, ND, 128).transpose(0, 3, 2, 1)
    )  # [B, 128, ND, PL]
    pt8c = PT.astype(NPF8)  # [B, 128, ND, PL]
    pwt8c = (PT * (wc.reshape(ND, 128).T[:, :, None] * 32.0)).astype(NPF8)
    # packed slot order [pt01, pwt01, pt23, pwt23]
    pq8 = np.concatenate(
        [pt8c[:, :, :2], pwt8c[:, :, :2], pt8c[:, :, 2:], pwt8c[:, :, 2:]], axis=2
    ).reshape(B, 128, 2 * ND * PL)
    if NW16:
        pt16 = (PT[:, :, :2] * 32.0).astype(NPBF).reshape(B, 128, 2 * PL)
    # row-block layout [B, 128, NI*D]: arr[b, p, i*D+k] = P[b, i*128+p, k]
    PN = np.ascontiguousarray(
        P.reshape(B, NI, 128, D).transpose(0, 2, 1, 3)
    ).reshape(B, 128, NI * D)
    pn8 = PN.astype(NPF8)
    sj = P @ wb  # [B, PL]
    sjt = np.ascontiguousarray(sj.reshape(B, NI, 128).transpose(0, 2, 1))

    W = np.stack([np.asarray(w, dtype=np.float32) for w in (w1, w2, w3)])  # [3, 2D, D]
    # per gate g: w16 holds P-chunks 0..nbf-1 plain (pt16 carries the x32);
    # w8 holds [P-chunks nbf..3 @ x32, attn-chunks 0-3 @ x4], tightly packed
    w16_parts, w8_parts = [], []
    for g in range(3):
        nbf = GATE_BF16[g]
        Wg = W[g].reshape(2 * ND, 128, D)  # contraction chunks
        w16_parts.append(Wg[:nbf].transpose(1, 0, 2))
        w8_parts.append(
            np.concatenate([Wg[nbf:ND] * 32.0, Wg[ND:] * 4.0], axis=0).transpose(1, 0, 2)
        )
    if NW16:
        w16 = np.ascontiguousarray(np.concatenate(w16_parts, axis=1)).astype(
            NPBF
        ).reshape(128, NW16 * D)
    w8 = np.ascontiguousarray(np.concatenate(w8_parts, axis=1)).astype(NPF8).reshape(
        128, NW8 * D
    )

    biases = np.stack([np.asarray(b, dtype=np.float32) for b in (b1, b2, b3)])
    with_bias = bool(np.any(biases))

    base = {"w8": w8}
    if NW16:
        base["w16"] = w16
    if with_bias:
        base["b32"] = biases * 32.0
    in_maps = []
    for c in range(NCORES):
        s = slice(c * BPC, (c + 1) * BPC)
        m = dict(base)
        m["pq8"] = np.ascontiguousarray(pq8[s])
        if NW16:
            m["pt16"] = pt16[s]
        m["pn8"] = pn8[s]
        m["pn32"] = PN[s]
        m["sjt"] = sjt[s]
        in_maps.append(m)
    return in_maps, with_bias


def run(P, w_atten, w1, w2, w3, b1, b2, b3, trace=False, taps=()):
    in_maps, with_bias = _prep_in_maps(P, w_atten, w1, w2, w3, b1, b2, b3)
    nc = _get_nc(with_bias, tuple(taps))
    res = run_bass_kernel_spmd(
        nc, in_maps, core_ids=list(range(NCORES)), trace=trace
    )
    out = np.concatenate([res.results[c]["out"] for c in range(NCORES)], axis=0)
    return out, res


def kernel(P, w_atten, w1, w2, w3, b1, b2, b3):
    out, _ = run(P, w_atten, w1, w2, w3, b1, b2, b3)
    return out


# revision 77
# speedup vs baseline: 1.0170x; 1.0170x over previous
"""Fused attention-encoding kernel for Trainium2, 8-core batch-parallel SPMD.

Problem (per batch b of 16, p=1024 tokens, d=512 features):
    A[i,j] = wa.P_i + wb.P_j + (wc*P_i).P_j        (si = wa.P_i cancels in softmax)
    SA     = softmax_j(A)
    attn   = SA @ P
    Pc     = [P, attn]
    out    = sigmoid(Pc@w2) * P + sigmoid(Pc@w3) * tanh(Pc@w1)

Strategy: batch-parallel over 8 cores (2 batches/core). Scores are computed
transposed (S^T[j,i], j on partitions) so sj folds into the exp as a
per-partition activation bias and the attention matmul consumes E=exp(S^T)
directly. The score/attention/rowsum matmuls run in fp8-e4m3 DoubleRow mode
(2 k-tiles per instruction); softmax protects them from quantization noise.
The gate matmuls are precision-graduated per gate (see GATE_MODE): the tanh
z-gate keeps half its P-contraction in bf16, the sigmoid gates run fully in
fp8 DoubleRow. All transposes, scale folds, and sj = P@wb are precomputed
host-side; inputs are shipped in SBUF-layout
([128 partitions, free]) so each tensor is one or two straight DMAs (DMA
triggers cost ~0.6us each on an engine ring, so fewer is faster). The softmax
reciprocal chain avoids the slow gpsimd partition-broadcast by broadcasting
the rowsum via a K=1 float32r matmul (full fp32 precision at bf16 speed).

Scale scheme (fp8-e4m3 wants operands ~O(1)):
    pwt8 = (P * wc * 32)^T   -> score PSUM is 32x, exp uses scale=1/32, bias=sjT
    ones = 1/8               -> rowsum PSUM = rs/8, so rb32 = 8/rs
    at8  = attn_unnorm * rb32 = 8*attn (fp8)
    pt16 = (P*32)^T bf16, w16 plain; w8 = w[512:]*4  -> gate PSUM is 32x logits,
    activations use scale=1/32 (bias b*32 added to PSUM before the rescale).
"""

import sys

if "/opt/trn_rl_repo" not in sys.path:
    sys.path.insert(0, "/opt/trn_rl_repo")

from contextlib import ExitStack

import ml_dtypes
import numpy as np

import concourse.bass as bass
import concourse.mybir as mybir
import concourse.tile as tile
from concourse import bacc
from concourse.bass_utils import run_bass_kernel_spmd

B, PL, D = 16, 1024, 512
NCORES = 8
BPC = B // NCORES          # batches per core
NI = PL // 128             # token blocks (i or j): 8
ND = D // 128              # feature chunks: 4
FP32 = mybir.dt.float32
FP32R = mybir.dt.float32r
BF16 = mybir.dt.bfloat16
FP8 = mybir.dt.float8e4
AF = mybir.ActivationFunctionType
DR = mybir.MatmulPerfMode.DoubleRow

NPF8 = ml_dtypes.float8_e4m3
NPBF = ml_dtypes.bfloat16

import os

# Per-gate P-half precision: how many of the 4 contraction chunks run in bf16
# (the rest run as fp8 DoubleRow pairs). The tanh z-gate amplifies logit error
# ~4x more than the sigmoids, so it keeps the bf16 chunks:
#   safe  (2,2,2): rel_err 1.12e-2   zsafe (2,0,0): 1.25e-2   full (0,0,0): 1.58e-2
GATE_MODE = os.environ.get("K_GATE_MODE", "zsafe")
GATE_BF16 = {"safe": (2, 2, 2), "zsafe": (2, 0, 0), "full": (0, 0, 0)}[GATE_MODE]
# per-gate chunk counts in w8 ([4-nbf P-chunks] + [4 attn chunks]) and offsets
W8_NCH = [8 - nbf for nbf in GATE_BF16]
W8_OFF = [sum(W8_NCH[:g]) for g in range(3)]
W16_OFF = [sum(GATE_BF16[:g]) for g in range(3)]
NW16 = sum(GATE_BF16)
NW8 = sum(W8_NCH)

_cache = {}


def _build(with_bias: bool, taps: tuple = ()):
    nc = bacc.Bacc(
        "TRN2", target_bir_lowering=False, debug=False, num_devices=1
    )
    # pt8/pwt8 packed as one tensor in chunk order [pt01, pwt01, pt23, pwt23]
    # so scores jb0's full operand set is one leading DMA (triggers ~0.6us each)
    pq8_d = nc.dram_tensor("pq8", [BPC, 128, 2 * ND * PL], FP8, kind="ExternalInput").ap()
    if NW16:
        pt16_d = nc.dram_tensor("pt16", [BPC, 128, 2 * PL], BF16, kind="ExternalInput").ap()
    pn8_d = nc.dram_tensor("pn8", [BPC, 128, NI * D], FP8, kind="ExternalInput").ap()
    pn32_d = nc.dram_tensor("pn32", [BPC, 128, NI * D], FP32, kind="ExternalInput").ap()
    sjt_d = nc.dram_tensor("sjt", [BPC, 128, NI], FP32, kind="ExternalInput").ap()
    if NW16:
        w16_d = nc.dram_tensor("w16", [128, NW16 * D], BF16, kind="ExternalInput").ap()
    w8_d = nc.dram_tensor("w8", [128, NW8 * D], FP8, kind="ExternalInput").ap()
    if with_bias:
        b_d = nc.dram_tensor("b32", [3, D], FP32, kind="ExternalInput").ap()
    out_d = nc.dram_tensor("out", [BPC, PL, D], FP32, kind="ExternalOutput").ap()
    tap_d = {}

    def tap(name, ap, lb=0):
        if lb != 0 or name not in taps:
            return
        t = nc.dram_tensor(
            f"tap_{name}", list(ap.shape), ap.dtype, kind="ExternalOutput"
        ).ap()
        tap_d[name] = t
        nc.sync.dma_start(t, ap)

    with tile.TileContext(nc) as tc, ExitStack() as ctx:
        pool = lambda name, bufs: ctx.enter_context(
            tc.tile_pool(name=name, bufs=bufs)
        )
        const = pool("const", 1)
        wpool = pool("wts", 1)
        pt8p = pool("pt8", 2)
        pt16p = pool("pt16", 2)
        pn8p = pool("pn8", 2)
        pn32p = pool("pn32", 2)
        e8p = pool("e8", 2)
        at8p = pool("at8", 2)
        rb32p = pool("rb32", 2)
        smallp = pool("small", 2)
        gp = pool("gates", 2)
        tmpp = pool("tmp", 2)
        op = pool("outs", 3)
        psmm = ctx.enter_context(tc.tile_pool(name="psmm", bufs=6, space="PSUM"))
        psvec = ctx.enter_context(tc.tile_pool(name="psvec", bufs=2, space="PSUM"))

        # --- constants / weights (loaded once, after batch-0 critical loads) ---
        if NW16:
            w16_sb = wpool.tile([128, NW16 * D], BF16, tag="w16")
        w8_sb = wpool.tile([128, NW8 * D], FP8, tag="w8")

        def load_weights():
            if NW16:
                nc.sync.dma_start(w16_sb[:], w16_d)
            nc.sync.dma_start(w8_sb[:], w8_d)

        # DoubleRow ldweights needs the k-tile pair step %16==0, so space the
        # two ones columns 16 elements apart.
        ones8 = const.tile([128, 32], FP8, tag="ones8")
        nc.vector.memset(ones8[:], 0.125)
        ones16 = const.tile([1, 128], BF16, tag="ones16")
        nc.vector.memset(ones16[:], 1.0)
        if with_bias:
            bb = [const.tile([128, D], FP32, tag=f"bias{g}", name=f"bias{g}") for g in range(3)]
            btmp = const.tile([1, 3 * D], FP32, tag="btmp")
            nc.sync.dma_start(btmp[:], b_d.rearrange("g e -> (g e)")[None, :])
            for g in range(3):
                nc.gpsimd.partition_broadcast(
                    bb[g][:], btmp[0:1, g * D : (g + 1) * D]
                )

        for lb in range(BPC):
            # ---------- phase A: loads (ring order = HBM priority) ----------
            pq8 = pt8p.tile([128, 2 * ND * PL], FP8, tag="pq8")
            for sl in (slice(0, 4 * PL), slice(4 * PL, 8 * PL)):
                nc.sync.dma_start(pq8[:, sl], pq8_d[lb][:, sl])
            sjt = smallp.tile([128, NI], FP32, tag="sjt")
            nc.scalar.dma_start(sjt[:], sjt_d[lb])
            pn8 = pn8p.tile([128, NI * D], FP8, tag="pn8")
            nc.sync.dma_start(pn8[:], pn8_d[lb])
            if NW16:
                pt16 = pt16p.tile([128, 2 * PL], BF16, tag="pt16")
                nc.sync.dma_start(pt16[:], pt16_d[lb])
            if lb == 0:
                load_weights()
            pn32 = pn32p.tile([128, NI * D], FP32, tag="pn32")
            nc.sync.dma_start(pn32[:], pn32_d[lb])

            pq8v = pq8.rearrange("p (c l) -> p c l", l=PL)
            PT_SLOT = (0, 4)  # packed slot of pt8 chunk-pair dp
            PW_SLOT = (2, 6)  # packed slot of pwt8 chunk-pair dp
            pn8v = pn8.rearrange("p (j d) -> p j d", d=D)

            # ---------- phase B: scores (fp8 DR) + exp + rowsum (fp8 DR) ----------
            e8 = e8p.tile([128, NI * PL], FP8, tag="e8")
            e8v = e8.rearrange("p (j l) -> p j l", l=PL)
            ps_rs = [
                psvec.tile([128, 512], FP32, tag="psvec", name=f"psrs{lb}_{_}")
                for _ in range(2)
            ]

            def rowsum(jb, start, stop):
                for ih in range(2):
                    nc.tensor.matmul(
                        ps_rs[ih][0:1, :],
                        ones8[:, 0:17:16][:, :, None],
                        e8v[:, jb - 1 : jb + 1, ih * 512 : (ih + 1) * 512],
                        start=start,
                        stop=stop,
                        perf_mode=DR,
                    )

            for jb in range(NI):
                ps_s = [
                    psmm.tile([128, 512], FP32, tag="psmm", name=f"pss{lb}_{jb}_{_}")
                    for _ in range(2)
                ]
                for ih in range(2):
                    for dp in range(2):
                        nc.tensor.matmul(
                            ps_s[ih],
                            pq8v[:, PT_SLOT[dp] : PT_SLOT[dp] + 2, jb * 128 : (jb + 1) * 128],
                            pq8v[:, PW_SLOT[dp] : PW_SLOT[dp] + 2, ih * 512 : (ih + 1) * 512],
                            start=(dp == 0),
                            stop=(dp == 1),
                            perf_mode=DR,
                        )
                for ih in range(2):
                    nc.scalar.activation(
                        e8v[:, jb, ih * 512 : (ih + 1) * 512],
                        ps_s[ih][:],
                        AF.Exp,
                        bias=sjt[:, jb : jb + 1],
                        scale=1.0 / 32.0,
                    )
                if jb % 2 == 1 and jb < NI - 1:
                    rowsum(jb, start=(jb == 1), stop=False)

            # ---------- phase C: attn^T (fp8 DR) + normalize (-> 8*attn fp8) ----------
            # dc0's first 3 jc-pairs only need exps jb0-5, so they run while
            # exp jb6/jb7 drain; the last rowsum pair and the final dc0 matmul
            # wait on exp jb7. The rowsum broadcast (K=1 fp32r matmul) slots in
            # right after so the reciprocal chain overlaps attn dc1-dc3.
            at8 = at8p.tile([128, ND * PL], FP8, tag="at8")
            at8v = at8.rearrange("p (c l) -> p c l", l=PL)
            rs16 = smallp.tile([1, PL], BF16, tag="rs16")
            rb32 = rb32p.tile([128, PL], FP32, tag="rb32")
            ps_bc = []

            def attn_mm(dc, ih, jp, ps_a):
                nc.tensor.matmul(
                    ps_a[ih],
                    pn8v[:, 2 * jp : 2 * jp + 2, dc * 128 : (dc + 1) * 128],
                    e8v[:, 2 * jp : 2 * jp + 2, ih * 512 : (ih + 1) * 512],
                    start=(jp == 0),
                    stop=(jp == 3),
                    perf_mode=DR,
                )

            # attn dc0/dc1 jc-pairs 0-2 need only exps jb0-5, so they run while
            # exp jb6/jb7 drain; the last rowsum pair and the final jc-pairs
            # wait on exp jb7. The rowsum broadcast (K=1 bf16 matmul) slots in
            # right after so the reciprocal chain overlaps attn dc2/dc3.
            ps_a = {}
            for dc in range(2):
                ps_a[dc] = [
                    psmm.tile([128, 512], FP32, tag="psmm", name=f"psa{lb}_{dc}_{_}")
                    for _ in range(2)
                ]
                for ih in range(2):
                    for jp in range(3):
                        attn_mm(dc, ih, jp, ps_a[dc])
            rowsum(NI - 1, start=False, stop=True)
            for ih in range(2):
                nc.scalar.copy(rs16[0:1, ih * 512 : (ih + 1) * 512], ps_rs[ih][0:1, :])
            for dc in range(2):
                for ih in range(2):
                    attn_mm(dc, ih, 3, ps_a[dc])
            for ih in range(2):
                bc = psvec.tile([128, 512], FP32, tag="psvec", name=f"psbc{lb}_{ih}")
                ps_bc.append(bc)
                nc.tensor.matmul(
                    bc[:],
                    ones16[:],
                    rs16[0:1, ih * 512 : (ih + 1) * 512],
                    start=True,
                    stop=True,
                )
            # interleave recip/normalize per half so the dc0 norm (which gates
            # PSUM reuse for attn dc3) lands one recip earlier
            for ih in range(2):
                nc.vector.reciprocal_approx_fast(
                    out=rb32[:, ih * 512 : (ih + 1) * 512], in_=ps_bc[ih][:]
                )
                nc.vector.tensor_mul(
                    at8v[:, 0, ih * 512 : (ih + 1) * 512],
                    ps_a[0][ih][:],
                    rb32[:, ih * 512 : (ih + 1) * 512],
                )
            for ih in range(2):
                nc.vector.tensor_mul(
                    at8v[:, 1, ih * 512 : (ih + 1) * 512],
                    ps_a[1][ih][:],
                    rb32[:, ih * 512 : (ih + 1) * 512],
                )
            for dc in range(2, ND):
                ps_ad = [
                    psmm.tile([128, 512], FP32, tag="psmm", name=f"psa{lb}_{dc}_{_}")
                    for _ in range(2)
                ]
                for ih in range(2):
                    for jp in range(4):
                        attn_mm(dc, ih, jp, ps_ad)
                for ih in range(2):
                    nc.vector.tensor_mul(
                        at8v[:, dc, ih * 512 : (ih + 1) * 512],
                        ps_ad[ih][:],
                        rb32[:, ih * 512 : (ih + 1) * 512],
                    )

            tap("sjt", sjt[:], lb)
            tap("e8", e8[:], lb)
            tap("rs32", rs16[:], lb)
            tap("at8", at8[:], lb)
            tap("w8_0", w8_sb[:, 0:2048], lb)

            # ---------- phase D: gates ----------
            # contraction: P chunks 0-1 bf16 (x32 P vs plain w), P chunks 2-3
            # as one fp8 DR pair (P vs 32w), attn chunks as two fp8 DR pairs
            # (8*attn vs 4w) -- every path lands 32x logits in PSUM.
            if NW16:
                pt16v = pt16.rearrange("p (c l) -> p c l", l=PL)
                w16v = w16_sb.rearrange("p (c d) -> p c d", d=D)
            w8v = w8_sb.rearrange("p (c d) -> p c d", d=D)
            for ib in range(NI):
                last = lb == BPC - 1 and ib == NI - 1
                ps_g = [
                    psmm.tile([128, 512], FP32, tag="psmm", name=f"psg{lb}_{ib}_{_}")
                    for _ in range(3)
                ]
                # final tile: z-gate matmuls last, so its PSUM stops right at
                # the end while r/f activations already ran -> shortest tail
                for g in ((1, 2, 0) if last else (0, 1, 2)):
                    nbf = GATE_BF16[g]
                    for dc in range(nbf):
                        nc.tensor.matmul(
                            ps_g[g],
                            pt16v[:, dc, ib * 128 : (ib + 1) * 128],
                            w16v[:, W16_OFF[g] + dc, :],
                            start=(dc == 0),
                            stop=False,
                        )
                    for dp in range(nbf // 2, 2):
                        nc.tensor.matmul(
                            ps_g[g],
                            pq8v[:, PT_SLOT[dp] : PT_SLOT[dp] + 2, ib * 128 : (ib + 1) * 128],
                            w8v[:, W8_OFF[g] + 2 * dp - nbf : W8_OFF[g] + 2 * dp - nbf + 2, :],
                            start=(nbf == 0 and dp == 0),
                            stop=False,
                            perf_mode=DR,
                        )
                    a0 = W8_OFF[g] + 4 - nbf
                    for cp in range(2):
                        nc.tensor.matmul(
                            ps_g[g],
                            at8v[:, 2 * cp : 2 * cp + 2, ib * 128 : (ib + 1) * 128],
                            w8v[:, a0 + 2 * cp : a0 + 2 + 2 * cp, :],
                            start=False,
                            stop=(cp == 1),
                            perf_mode=DR,
                        )
                if with_bias:
                    # bb holds b*32 so one 1/32 activation rescale covers both
                    for g in range(3):
                        nc.vector.tensor_add(ps_g[g][:], ps_g[g][:], bb[g][:])
                z32 = gp.tile([128, D], FP32, tag="z32")
                r32 = gp.tile([128, D], FP32, tag="r32")
                f32 = gp.tile([128, D], FP32, tag="f32")
                o32 = op.tile([128, D], FP32, tag="o32")
                t32 = tmpp.tile([128, D], FP32, tag="t32")
                sc = 1.0 / 32.0
                if not last:
                    # r first: the output chain is r -> o32 -> add -> DMA
                    nc.scalar.activation(r32[:], ps_g[1][:], AF.Sigmoid, scale=sc)
                    nc.scalar.activation(z32[:], ps_g[0][:], AF.Tanh, scale=sc)
                    nc.scalar.activation(f32[:], ps_g[2][:], AF.Sigmoid, scale=sc)
                    nc.vector.tensor_mul(o32[:], r32[:], pn32[:, ib * D : (ib + 1) * D])
                    # gpsimd offloads the f*z product off the vector engine
                    nc.gpsimd.tensor_mul(t32[:], f32[:], z32[:])
                    nc.vector.tensor_add(o32[:], o32[:], t32[:])
                    nc.gpsimd.dma_start(out_d[lb, ib * 128 : (ib + 1) * 128, :], o32[:])
                else:
                    # final tile: r/f acts + o32 overlap the z-gate matmuls;
                    # only z -> t -> add trails the last matmul. DMA triggers
                    # from the scalar ring so the slow gpsimd drain starts early
                    nc.scalar.activation(r32[:], ps_g[1][:], AF.Sigmoid, scale=sc)
                    nc.scalar.activation(f32[:], ps_g[2][:], AF.Sigmoid, scale=sc)
                    nc.scalar.activation(z32[:], ps_g[0][:], AF.Tanh, scale=sc)
                    nc.vector.tensor_mul(o32[:], r32[:], pn32[:, ib * D : (ib + 1) * D])
                    nc.vector.tensor_mul(t32[:], f32[:], z32[:])
                    nc.vector.tensor_add(o32[:], o32[:], t32[:])
                    nc.scalar.dma_start(out_d[lb, ib * 128 : (ib + 1) * 128, :], o32[:])

    nc.compile()
    return nc


def _get_nc(with_bias: bool, taps: tuple = ()):
    key = (with_bias, taps)
    if key not in _cache:
        _cache[key] = _build(with_bias, taps)
    return _cache[key]


def _prep_in_maps(P, w_atten, w1, w2, w3, b1, b2, b3):
    P = np.ascontiguousarray(np.asarray(P, dtype=np.float32))
    w_atten = np.asarray(w_atten, dtype=np.float32)
    wb = w_atten[D : 2 * D]
    wc = w_atten[2 * D :]

    # transposed layouts [B, 128, ND*PL]: arr[b, p, c*PL+l] = P[b, l, c*128+p]
    PT = np.ascontiguousarray(
        P.reshape(B, PL, ND, 128).transpose(0, 3, 2, 1)
    )  # [B, 128, ND, PL]
    pt8c = PT.astype(NPF8)  # [B, 128, ND, PL]
    pwt8c = (PT * (wc.reshape(ND, 128).T[:, :, None] * 32.0)).astype(NPF8)
    # packed slot order [pt01, pwt01, pt23, pwt23]
    pq8 = np.concatenate(
        [pt8c[:, :, :2], pwt8c[:, :, :2], pt8c[:, :, 2:], pwt8c[:, :, 2:]], axis=2
    ).reshape(B, 128, 2 * ND * PL)
    if NW16:
        pt16 = (PT[:, :, :2] * 32.0).astype(NPBF).reshape(B, 128, 2 * PL)
    # row-block layout [B, 128, NI*D]: arr[b, p, i*D+k] = P[b, i*128+p, k]
    PN = np.ascontiguousarray(
        P.reshape(B, NI, 128, D).transpose(0, 2, 1, 3)
    ).reshape(B, 128, NI * D)
    pn8 = PN.astype(NPF8)
    sj = P @ wb  # [B, PL]
    sjt = np.ascontiguousarray(sj.reshape(B, NI, 128).transpose(0, 2, 1))

    W = np.stack([np.asarray(w, dtype=np.float32) for w in (w1, w2, w3)])  # [3, 2D, D]
    # per gate g: w16 holds P-chunks 0..nbf-1 plain (pt16 carries the x32);
    # w8 holds [P-chunks nbf..3 @ x32, attn-chunks 0-3 @ x4], tightly packed
    w16_parts, w8_parts = [], []
    for g in range(3):
        nbf = GATE_BF16[g]
        Wg = W[g].reshape(2 * ND, 128, D)  # contraction chunks
        w16_parts.append(Wg[:nbf].transpose(1, 0, 2))
        w8_parts.append(
            np.concatenate([Wg[nbf:ND] * 32.0, Wg[ND:] * 4.0], axis=0).transpose(1, 0, 2)
        )
    if NW16:
        w16 = np.ascontiguousarray(np.concatenate(w16_parts, axis=1)).astype(
            NPBF
        ).reshape(128, NW16 * D)
    w8 = np.ascontiguousarray(np.concatenate(w8_parts, axis=1)).astype(NPF8).reshape(
        128, NW8 * D
    )

    biases = np.stack([np.asarray(b, dtype=np.float32) for b in (b1, b2, b3)])
    with_bias = bool(np.any(biases))

    base = {"w8": w8}
    if NW16:
        base["w16"] = w16
    if with_bias:
        base["b32"] = biases * 32.0
    in_maps = []
    for c in range(NCORES):
        s = slice(c * BPC, (c + 1) * BPC)
        m = dict(base)
        m["pq8"] = np.ascontiguousarray(pq8[s])
        if NW16:
            m["pt16"] = pt16[s]
        m["pn8"] = pn8[s]
        m["pn32"] = PN[s]
        m["sjt"] = sjt[s]
        in_maps.append(m)
    return in_maps, with_bias


def run(P, w_atten, w1, w2, w3, b1, b2, b3, trace=False, taps=()):
    in_maps, with_bias = _prep_in_maps(P, w_atten, w1, w2, w3, b1, b2, b3)
    nc = _get_nc(with_bias, tuple(taps))
    res = run_bass_kernel_spmd(
        nc, in_maps, core_ids=list(range(NCORES)), trace=trace
    )
    out = np.concatenate([res.results[c]["out"] for c in range(NCORES)], axis=0)
    return out, res


def kernel(P, w_atten, w1, w2, w3, b1, b2, b3):
    out, _ = run(P, w_atten, w1, w2, w3, b1, b2, b3)
    return out
